# revision 21
# baseline (speedup 1.0000x reference)
"""BackpropWiSARD embedding-lookup kernel for 8 Trainium2 NeuronCores.

Strategy (data-parallel over batch, table replicated):
  - Host: table (C,F,E) -> (F,E,C) bf16 padded to 128 classes, rows grouped
    in f-pairs (g, g+112) so each dma_gather call addresses a 16384-row
    window with int16 indices.
  - Each core handles B/8 = 64 batch rows.
  - Device per core:
      1. 56 scalar-offset indirect DMAs gather x^T rows by input_order ->
         mapped bits [p=f%112, (t,i), b]  (walrus supports only one dynamic
         offset per partition per indirect DMA).
      2. H3 hash on DVE: masked = mapped * hv[h,i] (int32), XOR-tree over i,
         XOR with t*8192 -> window-local row indices (int16 range).
      3. Index shuffle to dma_gather's int16 16-partition wrap layout via a
         DRAM round trip (8 strided cast DMAs out, 8 replica loads back).
      4. 112 dma_gather calls (512 idx each, 4 SWDGE queues): call g fetches
         rows of table window g into M[p=t*64+b, g, h, c].
      5. min over h, binarize (is_ge 0), tree-sum over g -> acc[p, c];
         selection-matrix matmul folds p=t*64+b partition pairs -> psum[b,c];
         affine 2S-F, add bias, DMA out (64,100) per core.
"""

import sys

sys.path.insert(0, "/opt/trn_rl_repo")

import numpy as np
import ml_dtypes

B, C, F, E, H, I = 512, 100, 224, 8192, 4, 28
NB = F * I  # 6272 input bits
NCORES = 8
BP = B // NCORES  # 64 batch rows per core
P1 = 112  # partitions carrying f % 112
T = F // P1  # 2
IP = 32  # i padded to power of two for the XOR tree
CP = 128  # classes padded for 256B gather rows
GW = T * E  # 16384-row table window per dma_gather call

_NC = None


def _build(loop_reps=1):
    import contextlib

    import concourse.bass as bass
    import concourse.mybir as mybir
    import concourse.tile as tile
    from concourse import bacc
    from concourse.library_config import mlp

    dt = mybir.dt
    op = mybir.AluOpType

    nc = bacc.Bacc(
        "TRN2", target_bir_lowering=False, debug=False, num_swdge_queues=4
    )

    tbl = nc.dram_tensor("tbl", (P1 * GW, CP), dt.bfloat16, kind="ExternalInput")
    xt = nc.dram_tensor("xt", (NB, BP), dt.int32, kind="ExternalInput")
    io = nc.dram_tensor("io", (P1, T * I), dt.int32, kind="ExternalInput")
    hvx = nc.dram_tensor("hvx", (P1, H * IP), dt.int32, kind="ExternalInput")
    rofs = nc.dram_tensor("rofs", (P1, T), dt.int32, kind="ExternalInput")
    sel = nc.dram_tensor("sel", (CP, BP), dt.bfloat16, kind="ExternalInput")
    biasx = nc.dram_tensor("biasx", (BP, C), dt.float32, kind="ExternalInput")
    outd = nc.dram_tensor("out", (BP, C), dt.float32, kind="ExternalOutput")

    with tile.TileContext(nc) as tc:
        nc.gpsimd.load_library(mlp)
        with (
            tc.tile_pool(name="main", bufs=1) as pool,
            tc.tile_pool(name="mc", bufs=2) as mpool,
            tc.tile_pool(name="dram", bufs=1, space="DRAM") as dpool,
            tc.tile_pool(name="psum", bufs=2, space="PSUM") as psum_pool,
            (tc.For_i(0, loop_reps, 1) if loop_reps > 1 else contextlib.nullcontext()),
        ):
            io_sb = pool.tile([P1, T * I], dt.int32)
            nc.sync.dma_start(out=io_sb[:], in_=io.ap())
            hvx_sb = pool.tile([P1, H, 1, IP, 1], dt.int32)
            nc.sync.dma_start(
                out=hvx_sb[:].rearrange("p h o i z -> p (h o i z)"), in_=hvx.ap()
            )
            bias_sb = pool.tile([BP, C], dt.float32)
            nc.sync.dma_start(out=bias_sb[:], in_=biasx.ap())
            sel_sb = pool.tile([CP, BP], dt.bfloat16)
            nc.sync.dma_start(out=sel_sb[:], in_=sel.ap())
            rowoff = pool.tile([P1, T, 1, 1], dt.int32)
            nc.sync.dma_start(
                out=rowoff[:].rearrange("p t o z -> p (t o z)"), in_=rofs.ap()
            )

            # mapped[p,(t,i),b] = xt[input_order[f*I+i], b],  f = t*P1+p
            mapped = pool.tile([P1, T * I, BP], dt.int32)
            for k in range(T * I):
                nc.gpsimd.indirect_dma_start(
                    out=mapped[:, k, :],
                    out_offset=None,
                    in_=xt.ap(),
                    in_offset=bass.IndirectOffsetOnAxis(ap=io_sb[:, k : k + 1], axis=0),
                )
            mapped4 = mapped[:].rearrange("p (t i) b -> p t i b", t=T)

            msk = pool.tile([P1, T, IP, BP], dt.int32)
            nc.vector.memset(msk[:, :, I:IP, :], 0)
            idxs = []
            for h in range(H):
                hv_h = hvx_sb[:, h, :, 0:I, :].to_broadcast([P1, T, I, BP])
                nc.vector.tensor_tensor(
                    out=msk[:, :, 0:I, :], in0=mapped4, in1=hv_h, op=op.mult
                )
                w = IP
                while w > 1:
                    w //= 2
                    nc.vector.tensor_tensor(
                        out=msk[:, :, 0:w, :],
                        in0=msk[:, :, 0:w, :],
                        in1=msk[:, :, w : 2 * w, :],
                        op=op.bitwise_xor,
                    )
                idx_h = pool.tile([P1, T, 1, BP], dt.int32, tag=f"idx{h}")
                ro = rowoff[:].to_broadcast([P1, T, 1, BP])
                nc.vector.tensor_tensor(
                    out=idx_h[:], in0=msk[:, :, 0:1, :], in1=ro, op=op.bitwise_xor
                )
                idxs.append(idx_h)

            # Shuffle indices into dma_gather's int16 wrap layout:
            # call g uses idx j = h*128 + t*64 + b (dst partition j%128 =
            # t*64+b, dst slot j//128 = h); idx tile position [j%16, j//16]
            # = [b%16, h*8 + t*4 + b//16].
            idx16 = []
            for h in range(H):
                i16 = pool.tile([P1, T, 4, 16], dt.int16, tag=f"i16_{h}")
                nc.vector.tensor_copy(
                    out=i16[:].rearrange("p t bh bl -> p (t bh bl)"),
                    in_=idxs[h][:].rearrange("p t o b -> p (t o b)"),
                )
                idx16.append(i16)
            dram_idx = dpool.tile([16, P1, H, T, 4], dt.int16)
            for h in range(H):
                for t in range(T):
                    for bh in range(4):
                        src = idx16[h][:, t : t + 1, bh : bh + 1, :].rearrange(
                            "p to bho bl -> p (to bho bl)"
                        )
                        dst = dram_idx[
                            :, :, h : h + 1, t : t + 1, bh : bh + 1
                        ].rearrange("bl g ho to bho -> g (ho to bho) bl")
                        nc.sync.dma_start(out=dst, in_=src)
            idxT = pool.tile([128, P1 * H * T * 4], dt.int16)
            dflat = dram_idx[:].rearrange("bl g h t bh -> bl (g h t bh)")
            for r in range(8):
                nc.sync.dma_start(out=idxT[r * 16 : (r + 1) * 16, :], in_=dflat)

            # Gather: call g fetches 512 rows from table window g into
            # Mc[p=t*64+b, gi, h, c]; then min over h, binarize, sum over g.
            acc = pool.tile([CP, 1, 1, CP], dt.bfloat16)
            nc.vector.memset(acc[:], 0)
            GC = 28
            for g0 in range(0, P1, GC):
                Mc = mpool.tile([128, GC, H, CP], dt.bfloat16, tag="Mc")
                for gi in range(GC):
                    g = g0 + gi
                    nc.gpsimd.dma_gather(
                        out_ap=Mc[:, gi, :, :],
                        in_ap=tbl.ap()[g * GW : (g + 1) * GW, :],
                        idxs_ap=idxT[:, g * 32 : (g + 1) * 32],
                        num_idxs=512,
                        num_idxs_reg=512,
                        elem_size=CP,
                        queue_num=0,
                    )
                nc.vector.tensor_tensor(
                    out=Mc[:, :, 0:1, :], in0=Mc[:, :, 0:1, :], in1=Mc[:, :, 1:2, :], op=op.min
                )
                nc.vector.tensor_tensor(
                    out=Mc[:, :, 2:3, :], in0=Mc[:, :, 2:3, :], in1=Mc[:, :, 3:4, :], op=op.min
                )
                nc.vector.tensor_tensor(
                    out=Mc[:, :, 0:1, :], in0=Mc[:, :, 0:1, :], in1=Mc[:, :, 2:3, :], op=op.min
                )
                nc.vector.tensor_scalar(
                    out=Mc[:, :, 0:1, :],
                    in0=Mc[:, :, 0:1, :],
                    scalar1=0.0,
                    scalar2=None,
                    op0=op.is_ge,
                )
                # tree-sum the GC {0,1} slabs, then accumulate
                w = GC
                while w > 1:
                    lo = w // 2
                    nc.vector.tensor_tensor(
                        out=Mc[:, 0:lo, 0:1, :],
                        in0=Mc[:, 0:lo, 0:1, :],
                        in1=Mc[:, lo : 2 * lo, 0:1, :],
                        op=op.add,
                    )
                    if w % 2:
                        nc.vector.tensor_tensor(
                            out=Mc[:, 0:1, 0:1, :],
                            in0=Mc[:, 0:1, 0:1, :],
                            in1=Mc[:, w - 1 : w, 0:1, :],
                            op=op.add,
                        )
                    w = lo
                nc.vector.tensor_tensor(
                    out=acc[:], in0=acc[:], in1=Mc[:, 0:1, 0:1, :], op=op.add
                )

            # fold p = t*64+b partition pairs: psum[b,c] = sum_p sel[p,b]*acc[p,c]
            S = psum_pool.tile([BP, CP], dt.float32, tag="S")
            nc.tensor.matmul(
                out=S[:],
                lhsT=sel_sb[:],
                rhs=acc[:].rearrange("p o z c -> p (o z c)"),
                start=True,
                stop=True,
            )
            res = pool.tile([BP, C], dt.float32)
            nc.vector.tensor_scalar(
                out=res[:],
                in0=S[:, 0:C],
                scalar1=2.0,
                scalar2=float(-F),
                op0=op.mult,
                op1=op.add,
            )
            nc.vector.tensor_tensor(out=res[:], in0=res[:], in1=bias_sb[:], op=op.add)
            nc.sync.dma_start(out=outd.ap(), in_=res[:])

    nc.compile()
    return nc


def get_nc(loop_reps=1):
    global _NC
    if loop_reps != 1:
        return _build(loop_reps)
    if _NC is None:
        _NC = _build()
    return _NC


def prep_in_maps(inputs):
    x_b = np.asarray(inputs["x_b"], dtype=np.int32)
    input_order = np.asarray(inputs["input_order"], dtype=np.int32)
    hash_values = np.asarray(inputs["hash_values"], dtype=np.int32)
    table = np.asarray(inputs["table"], dtype=np.float32)
    bias = np.asarray(inputs["bias"], dtype=np.float32)

    # (C,F,E) -> (F,E,CP) bf16 rows (sign-preserving cast; pad classes with
    # zeros), then group rows so window g holds f = g and f = g+112.
    tp = np.zeros((F, E, CP), dtype=ml_dtypes.bfloat16)
    tp[:, :, :C] = table.transpose(1, 2, 0).astype(ml_dtypes.bfloat16)
    tt = np.ascontiguousarray(
        tp.reshape(T, P1, E, CP).transpose(1, 0, 2, 3)
    ).reshape(P1 * GW, CP)

    io_arr = np.ascontiguousarray(
        input_order.reshape(T, P1, I).transpose(1, 0, 2)
    ).reshape(P1, T * I)

    hvx = np.zeros((P1, H, IP), dtype=np.int32)
    hvx[:, :, :I] = hash_values[None, :, :]
    hvx = hvx.reshape(P1, H * IP)

    rofs = np.broadcast_to(
        np.arange(T, dtype=np.int32)[None, :] * E, (P1, T)
    ).astype(np.int32)

    selm = np.tile(np.eye(BP, dtype=np.float32), (T, 1)).astype(ml_dtypes.bfloat16)
    selm = np.ascontiguousarray(selm)  # (128, 64)

    biasx = np.ascontiguousarray(np.tile(bias.reshape(1, C), (BP, 1)))

    in_maps = []
    for k in range(NCORES):
        xtk = np.ascontiguousarray(x_b[k * BP : (k + 1) * BP].T)
        in_maps.append(
            {
                "tbl": tt,
                "xt": xtk,
                "io": io_arr,
                "hvx": hvx,
                "rofs": rofs,
                "sel": selm,
                "biasx": biasx,
            }
        )
    return in_maps


def kernel(**inputs):
    from concourse.bass_utils import run_bass_kernel_spmd

    nc = get_nc()
    in_maps = prep_in_maps(inputs)
    res = run_bass_kernel_spmd(nc, in_maps, list(range(NCORES)))
    parts = [res.results[k]["out"].reshape(BP, C) for k in range(NCORES)]
    return np.concatenate(parts, axis=0).astype(np.float32)
